# revision 58
# baseline (speedup 1.0000x reference)
"""Multi-head cross-attention Trainium2 kernel (8-core SPMD, data-parallel).

Shards (batch=4) x (seq halves) across 8 NeuronCores; each core runs the
full q/kv/attention/out-proj pipeline for its 2048 query rows in bf16 with
fp32 PSUM accumulation.

Key tricks:
  - mask: reference adds +1.0 to logits of keys j < mask[b] before softmax.
    softmax(l + m) = e^m * e^l / sum  ->  fold e^m into V rows (and into the
    softmax-sum ones column), so masking costs nothing per tile.
  - attn@V is computed with exp-weights as the stationary operand and V as
    the moving operand, so the head output lands naturally as [query, dim]:
    normalization is then a per-partition tensor_scalar (no transposes).
  - softmax sums come from an extra ones column appended to V (head_dim 73).
  - per-head K^T tiles are zero-padded to full 128-partition chunks so every
    matmul operand sits at base partition 0 (tile_position constraint).
  - logits accumulate into 2-bank PSUM tiles so exp runs as 2 activation
    instructions per head instead of 3 (Act engine is the head-loop limiter);
    the attn@V accumulator reuses the logits tile's PSUM banks.
  - q-proj of group g+1 and out-proj of group g-1 are interleaved into group
    g's head loop so exp latency hides under PE work.
  - output is written bf16 and upcast to f32 on the host (halves out DMA).
"""

import sys

sys.path.insert(0, "/opt/trn_rl_repo")

import ml_dtypes
import numpy as np

import concourse.bass as bass  # noqa: F401  (engine types via nc)
import concourse.mybir as mybir
import concourse.tile as tile
from concourse import bacc
from concourse.bass_utils import run_bass_kernel_spmd
from concourse.masks import make_identity

BF16 = mybir.dt.bfloat16
F32 = mybir.dt.float32
NPBF16 = ml_dtypes.bfloat16
AF = mybir.ActivationFunctionType

B, NSEQ, MKEY, D, H, DH = 4, 4096, 300, 1152, 16, 72
NCORES = 8
C = D // 128  # 9 feature chunks
KC = 3  # key chunks, keys padded 300 -> 384
MP = KC * 128
RG = 512  # query rows per group
TPG = RG // 128  # 128-row tiles per group
SCALE = 1.0 / float(np.sqrt(DH))
ROWS_PER_CORE = B * NSEQ // NCORES  # 2048

LAST_EXEC_NS = None


def _head_segs(h):
    """Feature range [72h, 72h+72) of head h split at 128-chunk boundaries.

    Returns [(chunk, lo, hi)] with chunk-local partition range [lo, hi)."""
    f0, f1 = DH * h, DH * h + DH
    segs = []
    c = f0 // 128
    while c * 128 < f1:
        lo = max(f0, c * 128) - c * 128
        hi = min(f1, (c + 1) * 128) - c * 128
        segs.append((c, lo, hi))
        c += 1
    return segs


def _chunk_segs(c):
    """[(h, i, lo, hi)] head segments living in feature chunk c."""
    out = []
    for h in range(H):
        for i, (hc, lo, hi) in enumerate(_head_segs(h)):
            if hc == c:
                out.append((h, i, lo, hi))
    return out


# flat order of all (head, segment) pairs; column index into the hmask input
_ALL_SEGS = [(h, i) for h in range(H) for i in range(len(_head_segs(h)))]
_SEG_IDX = {hs: s for s, hs in enumerate(_ALL_SEGS)}
NSEG = len(_ALL_SEGS)


def _hmask_host():
    """[128, NSEG] f32: column (h,i) is 1.0 on the chunk-local partitions of
    that head segment, 0 elsewhere. Engine ops can't address SBUF at
    non-32-aligned partition bases, so head extraction is done as a
    full-chunk copy multiplied by this per-partition mask."""
    m = np.zeros((128, NSEG), np.float32)
    for h in range(H):
        for i, (_, lo, hi) in enumerate(_head_segs(h)):
            m[lo:hi, _SEG_IDX[(h, i)]] = 1.0
    return m


def build_program(rpc=ROWS_PER_CORE, has_bq=False, has_bk=False, has_bv=False, has_bp=False):
    nc = bacc.Bacc()

    xT_d = nc.dram_tensor("xT", [C, 128, rpc], BF16, kind="ExternalInput")
    condT_d = nc.dram_tensor("condT", [C, 128, MKEY], BF16, kind="ExternalInput")
    # wq is laid out c_out-major: wq_d[c] holds Wq[:, c*128:(c+1)*128] as
    # [128 rows of each k-chunk, (k, j)] so the first q-proj chunk only needs
    # one 294KB DMA instead of the whole 2.65MB weight.
    wq_d = nc.dram_tensor("wq", [C, 128, D], BF16, kind="ExternalInput")
    wk_d = nc.dram_tensor("wk", [C, 128, D], BF16, kind="ExternalInput")
    wv_d = nc.dram_tensor("wv", [C, 128, D], BF16, kind="ExternalInput")
    wp_d = nc.dram_tensor("wp", [C, 128, D], BF16, kind="ExternalInput")
    bq_d = nc.dram_tensor("bq", [128, C], F32, kind="ExternalInput")
    bk_d = nc.dram_tensor("bk", [128, C], F32, kind="ExternalInput")
    bv_d = nc.dram_tensor("bv", [1, D], BF16, kind="ExternalInput")
    bp_d = nc.dram_tensor("bp", [1, D], BF16, kind="ExternalInput")
    vs_d = nc.dram_tensor("vscale", [128, KC], F32, kind="ExternalInput")
    hm_d = nc.dram_tensor("hmask", [128, NSEG], F32, kind="ExternalInput")
    out_d = nc.dram_tensor("out", [rpc, D], BF16, kind="ExternalOutput")

    groups = rpc // RG
    kn = [128, 128, MKEY - 256]  # real keys per key chunk
    single = {h: _head_segs(h)[0] for h in range(H) if len(_head_segs(h)) == 1}
    straddle = [h for h in range(H) if h not in single]
    head_order = sorted(single) + straddle

    with tile.TileContext(nc) as tc:
        with (
            tc.tile_pool(name="const", bufs=1) as cpool,
            tc.tile_pool(name="ps", bufs=2, space="PSUM") as ps,
            tc.tile_pool(name="xq", bufs=2) as xq_pool,
        ):
            # ---- input DMAs, ordered so the first q-proj chunk starts ASAP
            wq_sb = cpool.tile([128, C, C, 128], BF16)  # [p, c_out, k, j]
            nc.sync.dma_start(wq_sb[:, 0, 0, :], wq_d[0][:, 0:128])
            nc.sync.dma_start(wq_sb[:, 0, 1:, :], wq_d[0][:, 128:D])

            xts, qts = {}, {}

            def x_dma(g, eng=None, split=False):
                # startup loads ride the idle Activation HWDGE queue so they
                # run in parallel with the weight DMAs on the sync queue;
                # mid-kernel prefetches go through gpsimd's software DGE.
                eng = eng or nc.gpsimd
                xT_sb = xq_pool.tile([128, C, RG], BF16, name="xT", tag="xT")
                rr = [nc.scalar, nc.sync, nc.gpsimd]
                for k in range(C):
                    e = rr[k % 3] if split else eng
                    e.dma_start(xT_sb[:, k, :], xT_d[k][:, g * RG : (g + 1) * RG])
                xts[g] = xT_sb
                qts[g] = xq_pool.tile([128, C, RG], BF16, name="qT", tag="qT")

            x_dma(0, nc.scalar, split=True)
            for c in range(1, C):
                nc.sync.dma_start(wq_sb[:, c, :, :], wq_d[c])

            wp_sb = cpool.tile([128, C, D], BF16)

            ident = cpool.tile([128, 128], BF16)
            make_identity(nc, ident[:])
            vs_sb = cpool.tile([128, KC], F32)
            nc.sync.dma_start(vs_sb[:], vs_d[:])
            hm_sb = cpool.tile([128, NSEG], F32)
            nc.sync.dma_start(hm_sb[:], hm_d[:])
            ones16 = cpool.tile([128, 16], BF16)
            nc.gpsimd.memset(ones16[:], 1.0)
            if has_bq:
                bq_sb = cpool.tile([128, C], F32)
                nc.sync.dma_start(bq_sb[:], bq_d[:])
            if has_bk:
                bk_sb = cpool.tile([128, C], F32)
                nc.sync.dma_start(bk_sb[:], bk_d[:])
            if has_bp:
                bp_sb = cpool.tile([1, D], BF16)
                nc.sync.dma_start(bp_sb[:], bp_d[:])
            if has_bv or has_bp:
                ones_sb = cpool.tile([1, 128], BF16)
                nc.gpsimd.memset(ones_sb[:], 1.0)

            # V in natural orientation [key, head, dim+1]; fake keys stay 0,
            # col 72 holds e^mask (ones column pre-scaled by the mask factor)
            v_sb = cpool.tile([128, KC, H, DH + 1], BF16)
            nc.gpsimd.memset(v_sb[:], 0.0)
            kTz = {}
            for h in range(H):
                for i in range(len(_head_segs(h))):
                    t = cpool.tile([128, MP], BF16, name=f"kTz_{h}_{i}")
                    if h in single:
                        nc.gpsimd.memset(t[:, MKEY:MP], 0.0)
                    kTz[(h, i)] = t
            kT72 = {}
            for h in straddle:
                t = cpool.tile([DH + 4, MP], BF16, name=f"kT72_{h}")
                nc.gpsimd.memset(t[0:DH, MKEY:MP], 0.0)
                kT72[h] = t

            # ---- q-proj chunk emitter (interleaved into head loops) ----
            def qp_chunk(g, c):
                qps = ps.tile([128, RG], F32, name="qps", tag="sm")
                for k in range(C):
                    nc.tensor.matmul(
                        qps[:],
                        wq_sb[:, c, k, :],
                        xts[g][:, k, :],
                        start=(k == 0),
                        stop=(k == C - 1),
                    )
                if has_bq:
                    nc.scalar.activation(
                        qts[g][:, c, :], qps[:], AF.Identity, bias=bq_sb[:, c : c + 1]
                    )
                else:
                    nc.vector.tensor_copy(qts[g][:, c, :], qps[:])

            # ---- kv prologue: K/V projections (weights in a scoped pool) ----
            with tc.tile_pool(name="kvw", bufs=1) as kv_pool:
                condT_sb = kv_pool.tile([128, C, MKEY], BF16)
                wk_sb = kv_pool.tile([128, C, D], BF16)
                wv_sb = kv_pool.tile([128, C, D], BF16)
                for k in range(C):
                    nc.scalar.dma_start(condT_sb[:, k, :], condT_d[k])
                for k in range(C):
                    nc.sync.dma_start(wk_sb[:, k, :], wk_d[k])
                x_dma(1, nc.scalar)
                for k in range(C):
                    nc.sync.dma_start(wv_sb[:, k, :], wv_d[k])
                for k in range(C):
                    nc.sync.dma_start(wp_sb[:, k, :], wp_d[k])
                if has_bv:
                    bv_sb = kv_pool.tile([1, D], BF16)
                    nc.sync.dma_start(bv_sb[:], bv_d[:])

                for c in range(C):
                    qp_chunk(0, c)

                # K projection -> zero-padded per-head K^T tiles
                for c in range(C):
                    kps = ps.tile([128, MKEY], F32, name="kps", tag="sm")
                    for k in range(C):
                        nc.tensor.matmul(
                            kps[:],
                            wk_sb[:, k, c * 128 : (c + 1) * 128],
                            condT_sb[:, k, :],
                            start=(k == 0),
                            stop=(k == C - 1),
                        )
                    for h, i, _lo, _hi in _chunk_segs(c):
                        s = _SEG_IDX[(h, i)]
                        if has_bk:
                            nc.vector.tensor_scalar(
                                kTz[(h, i)][:, 0:MKEY],
                                kps[:],
                                bk_sb[:, c : c + 1],
                                hm_sb[:, s : s + 1],
                                op0=mybir.AluOpType.add,
                                op1=mybir.AluOpType.mult,
                            )
                        else:
                            nc.vector.tensor_scalar_mul(
                                kTz[(h, i)][:, 0:MKEY], kps[:], hm_sb[:, s : s + 1]
                            )
                for h, t in kT72.items():
                    (c0, lo0, hi0), (c1, lo1, hi1) = _head_segs(h)
                    n0 = hi0 - lo0
                    nc.gpsimd.dma_start(t[0:n0, 0:MKEY], kTz[(h, 0)][lo0:hi0, 0:MKEY])
                    nc.gpsimd.dma_start(
                        t[n0 : n0 + (hi1 - lo1), 0:MKEY], kTz[(h, 1)][lo1:hi1, 0:MKEY]
                    )

                # V projection (natural orientation, head-aligned chunks)
                vch = [(0, 360), (360, 720), (720, 1080), (1080, 1152)]
                for f0, f1 in vch:
                    for kc in range(KC):
                        vps = ps.tile([128, 360], F32, name="vps", tag="sm")
                        for k in range(C):
                            nc.tensor.matmul(
                                vps[0 : kn[kc], 0 : f1 - f0],
                                condT_sb[:, k, kc * 128 : kc * 128 + kn[kc]],
                                wv_sb[:, k, f0:f1],
                                start=(k == 0),
                                stop=(k == C - 1 and not has_bv),
                            )
                        if has_bv:
                            nc.tensor.matmul(
                                vps[0 : kn[kc], 0 : f1 - f0],
                                ones_sb[0:1, 0 : kn[kc]],
                                bv_sb[0:1, f0:f1],
                                start=False,
                                stop=True,
                            )
                        for h in range(f0 // DH, f1 // DH):
                            d0 = h * DH - f0
                            nc.vector.tensor_scalar_mul(
                                v_sb[0 : kn[kc], kc, h, 0:DH],
                                vps[0 : kn[kc], d0 : d0 + DH],
                                vs_sb[0 : kn[kc], kc : kc + 1],
                            )
                        # ones column (pre-scaled by mask factor), these heads
                        h0, h1 = f0 // DH, f1 // DH
                        nc.vector.tensor_scalar_mul(
                            v_sb[0 : kn[kc], kc, h0:h1, DH],
                            ones16[0 : kn[kc], 0 : h1 - h0],
                            vs_sb[0 : kn[kc], kc : kc + 1],
                        )

            # ---- attention ----
            att_cm = tc.tile_pool(name="att", bufs=1)
            outp_cm = tc.tile_pool(name="outp", bufs=2)
            att_pool = att_cm.__enter__()
            out_pool = outp_cm.__enter__()

            def tail_a(g, rt, onat_sb, oTc_sb, c3):
                """transpose 3 onat chunks -> oTc (feature-major) for out-proj"""
                # the last group's tail runs after the head loops: transposes
                # borrow the then-idle 3-bank logits ring while out-proj keeps
                # the sm ring, so the two chains don't share one ring
                tag = "lps" if g == groups - 1 else "sm"
                tps = ps.tile([128, 3, 128], BF16, name="tps", tag=tag)
                for j in range(3):
                    c = c3 * 3 + j
                    nc.tensor.transpose(
                        tps[:, j, :], onat_sb[:, c * 128 : (c + 1) * 128], ident[:]
                    )
                nc.vector.tensor_copy(oTc_sb[:, c3 * 3 : c3 * 3 + 3, :], tps[:])

            def tail_b(g, rt, oTc_sb, fi):
                grt = g * TPG + rt
                f0, f1 = [(0, 384), (384, 768), (768, 1152)][fi]
                if fi == 0:
                    ysbs[(g, rt)] = out_pool.tile([128, D], BF16, name="ysb", tag="y", bufs=3)
                ysb = ysbs[(g, rt)]
                yps = ps.tile([128, 384], F32, name="yps", tag="sm")
                for c in range(C):
                    nc.tensor.matmul(
                        yps[:],
                        oTc_sb[:, c, :],
                        wp_sb[:, c, f0:f1],
                        start=(c == 0),
                        stop=(c == C - 1 and not has_bp),
                    )
                if has_bp:
                    nc.tensor.matmul(
                        yps[:],
                        ones_sb[0:1, :],
                        bp_sb[0:1, f0:f1],
                        start=False,
                        stop=True,
                    )
                nc.vector.tensor_copy(ysb[:, f0:f1], yps[:])
                if fi == 2:
                    nc.sync.dma_start(out_d[grt * 128 : (grt + 1) * 128, :], ysb[:])

            ysbs = {}

            def make_tail_items(g, onats):
                items = []
                oTcs = [
                    out_pool.tile([128, C, 128], BF16, name=f"oTc{rt}", tag=f"oTc{rt}")
                    for rt in range(TPG)
                ]
                for rt in range(TPG):
                    for c3 in range(C // 3):
                        items.append(
                            (0.25, lambda g=g, rt=rt, o=onats[rt], t=oTcs[rt], c3=c3: tail_a(g, rt, o, t, c3))
                        )
                    for fi in range(3):
                        items.append(
                            (1.45, lambda g=g, rt=rt, t=oTcs[rt], fi=fi: tail_b(g, rt, t, fi))
                        )
                return items

            def head_loop(g, items):
                """QK -> exp -> AV -> normalize for all heads of group g,
                with independent work items interleaved to hide exp latency."""
                qT_sb = qts[g]
                qTg = {}
                for h in straddle:
                    (c0, lo0, hi0), (c1, lo1, hi1) = _head_segs(h)
                    n0 = hi0 - lo0
                    t = out_pool.tile([DH + 4, RG], BF16, name=f"qTg{h}", tag=f"qTg{h}")
                    nc.gpsimd.dma_start(t[0:n0, :], qT_sb[lo0:hi0, c0, :])
                    nc.gpsimd.dma_start(t[n0:DH, :], qT_sb[lo1:hi1, c1, :])
                    qTg[h] = t

                onats = [
                    out_pool.tile([128, D], BF16, name=f"onat{rt}", tag=f"onat{rt}")
                    for rt in range(TPG)
                ]

                def qk(h):
                    lps = ps.tile([128, KC, RG], F32, name="lps", tag="lps")
                    for kc in range(KC):
                        if h in single:
                            c, _lo, _hi = single[h]
                            nc.tensor.matmul(
                                lps[:, kc, :],
                                kTz[(h, 0)][:, kc * 128 : (kc + 1) * 128],
                                qT_sb[:, c, :],
                                start=True,
                                stop=True,
                            )
                        else:
                            nc.tensor.matmul(
                                lps[:, kc, :],
                                kT72[h][0:DH, kc * 128 : (kc + 1) * 128],
                                qTg[h][0:DH, :],
                                start=True,
                                stop=True,
                            )
                    expT = att_pool.tile([128, KC, RG], BF16, name="expT", tag="expT", bufs=5)
                    nc.scalar.activation(expT[:], lps[:], AF.Exp, scale=SCALE)
                    # attn@V accumulator reuses the (now-consumed) logits banks
                    return expT, lps

                def av(h, expT, lps):
                    for qt in range(TPG):
                        acc = lps[:, 0, qt * 76 : qt * 76 + DH + 1]
                        for kc in range(KC):
                            nc.tensor.matmul(
                                acc,
                                expT[:, kc, qt * 128 : (qt + 1) * 128],
                                v_sb[:, kc, h, :],
                                start=(kc == 0),
                                stop=(kc == KC - 1),
                            )
                    # one copy releases the logits PSUM tile early; normalize
                    # then runs out of SBUF off the PE-critical ring
                    oc = att_pool.tile([128, TPG * 76], F32, name="oc", tag="oc", bufs=4)
                    nc.vector.tensor_copy(oc[:], lps[:, 0, 0 : TPG * 76])
                    inv = att_pool.tile([128, TPG], F32, name="inv", tag="inv", bufs=4)
                    nc.vector.reciprocal(inv[:], oc[:, DH : TPG * 76 : 76])
                    for qt in range(TPG):
                        nc.vector.tensor_scalar_mul(
                            onats[qt][:, h * DH : (h + 1) * DH],
                            oc[:, qt * 76 : qt * 76 + DH],
                            inv[:, qt : qt + 1],
                        )

                # spread interleaved items evenly (by PE-cost) across heads
                total = sum(c for c, _ in items)
                prev = None
                it = 0
                cum = 0.0
                for i, h in enumerate(head_order):
                    cur = (h,) + qk(h)
                    if prev is not None:
                        av(*prev)
                    while it < len(items) and cum < (i + 1) * total / H:
                        cum += items[it][0]
                        items[it][1]()
                        it += 1
                    prev = cur
                av(*prev)
                while it < len(items):
                    items[it][1]()
                    it += 1
                return onats

            tails = []
            hold = []
            for g in range(groups):
                items = []
                if g + 1 < groups:
                    if g + 2 < groups:
                        items.append((0.0, lambda g=g: x_dma(g + 2)))
                    items += [(1.9, lambda g=g, c=c: qp_chunk(g + 1, c)) for c in range(C)]
                avail = hold + tails
                if g == groups - 2:
                    # the last head loop has no q-proj filler: defer some of
                    # this round's out-proj work into it to keep PE ahead of exp
                    items += avail[:-6]
                    hold = avail[-6:]
                else:
                    items += avail
                    hold = []
                onats = head_loop(g, items)
                tails = make_tail_items(g, onats)
            for _c, t in hold + tails:
                t()

            outp_cm.__exit__(None, None, None)
            att_cm.__exit__(None, None, None)

    nc.compile()
    return nc


_programs = {}


def _get_program(key):
    if key not in _programs:
        _programs[key] = build_program(*key)
    return _programs[key]


def make_in_maps(x, cond, mask, Wq, bq, Wkv, bkv, Wp, bp, rpc=ROWS_PER_CORE, ncores=NCORES):
    """Host-side shard + relayout. Returns (in_maps, flags)."""
    x = np.asarray(x, np.float32)
    cond = np.asarray(cond, np.float32)
    mask = np.asarray(mask)
    Wq = np.asarray(Wq, np.float32)
    Wkv = np.asarray(Wkv, np.float32)
    Wp = np.asarray(Wp, np.float32)
    bq = np.asarray(bq, np.float32)
    bkv = np.asarray(bkv, np.float32)
    bp = np.asarray(bp, np.float32)

    # wq c_out-major: wq[c][p][k*128+j] = Wq[k*128+p, c*128+j]
    wq = np.ascontiguousarray(
        Wq.astype(NPBF16).reshape(C, 128, C, 128).transpose(2, 1, 0, 3).reshape(C, 128, D)
    )
    wk = np.ascontiguousarray(Wkv[:, :D].astype(NPBF16).reshape(C, 128, D))
    wv = np.ascontiguousarray(Wkv[:, D:].astype(NPBF16).reshape(C, 128, D))
    wp = np.ascontiguousarray(Wp.astype(NPBF16).reshape(C, 128, D))
    bq_a = np.ascontiguousarray(bq.reshape(C, 128).T)
    bk_a = np.ascontiguousarray(bkv[:D].reshape(C, 128).T)
    bv_a = bkv[D:].astype(NPBF16).reshape(1, D)
    bp_a = bp.astype(NPBF16).reshape(1, D)

    flags = (rpc, bool(bq.any()), bool(bkv[:D].any()), bool(bkv[D:].any()), bool(bp.any()))
    hmask = _hmask_host()

    halves = NSEQ // rpc
    in_maps = []
    for core in range(ncores):
        b, half = core // halves, core % halves
        rows = slice(half * rpc, (half + 1) * rpc)
        xT = np.ascontiguousarray(x[b, rows].T.astype(NPBF16)).reshape(C, 128, rpc)
        condT = np.ascontiguousarray(cond[b].T.astype(NPBF16)).reshape(C, 128, MKEY)
        mv = (np.arange(MP) < int(mask[b])).astype(np.float32)
        vscale = np.ascontiguousarray(np.exp(mv).reshape(KC, 128).T)
        in_maps.append(
            {
                "xT": xT,
                "condT": condT,
                "wq": wq,
                "wk": wk,
                "wv": wv,
                "wp": wp,
                "bq": bq_a,
                "bk": bk_a,
                "bv": bv_a,
                "bp": bp_a,
                "vscale": vscale,
                "hmask": hmask,
            }
        )
    return in_maps, flags


def kernel(x, cond, mask, Wq, bq, Wkv, bkv, Wp, bp):
    global LAST_EXEC_NS
    import os
    import time

    in_maps, flags = make_in_maps(x, cond, mask, Wq, bq, Wkv, bkv, Wp, bp)
    nc = _get_program(flags)
    trace = bool(os.environ.get("BASS_KERNEL_TRACE"))
    res = None
    for attempt in range(3):
        try:
            res = run_bass_kernel_spmd(nc, in_maps, list(range(NCORES)), trace=trace)
            break
        except Exception:
            if attempt == 2:
                raise
            time.sleep(10)
    LAST_EXEC_NS = res.exec_time_ns

    rpc = flags[0]
    halves = NSEQ // rpc
    out = np.empty((B, NSEQ, D), np.float32)
    for core in range(NCORES):
        b, half = core // halves, core % halves
        out[b, half * rpc : (half + 1) * rpc] = res.results[core]["out"].astype(np.float32)
    return out
